# revision 21
# baseline (speedup 1.0000x reference)
"""PersistenceLandscapeLoss on 8 TRN2 NeuronCores via Bass/Tile.

Math (reference):
  D[i,j] = ||e_i - e_j||          (i != j; diag pushed to 1e9)
  d_min/d_max = min/max off-diag; thresholds = linspace(d_min, max(d_max, d_min+1e-4), 24)
  per threshold t: adj = sigmoid((t - D)/0.15) (zero diag); deg_i = row sums
  h0_t = #(deg_i < 0.5); S_t = sum(adj); n_excess_t = relu(S_t/2 - (N-1))/N
  loss = (mean(h0[-8:]) + 0.5*mean(n_excess)) * 0.1

Sharding: 512 distance-matrix rows per core; columns permuted per-core so the
diagonal block is at a static position (core's own columns first).
Per core:
  - GEMM on PE in bf16 hi/lo split (3 passes ~ fp32 accuracy, 2.5x faster
    than native fp32 matmul): psum = -2*G; DVE adds sq_i + sq_j; ACT sqrt.
  - EC = exp(-(D-c0)/TEMP) (bf16) precomputed on ACT during the GEMM phase;
    used by the DVE sigmoid path (sigmoid(x) = 1 - 1/(1+e^x)).
  - row min/max on DVE; AllGather + cross-lane max for global d_min/d_max;
    thresholds on-device mirroring jnp.linspace fp32 semantics.
  - 24 thresholds: first XDVE on DVE (mul+add, reciprocal_approx_fast,
    reduce), rest on ACT Sigmoid with accum_out. The two engines run
    concurrently.
Host gathers per-row degree partials and finishes the scalar reduction.
"""
import sys

if "/opt/trn_rl_repo" not in sys.path:
    sys.path.insert(0, "/opt/trn_rl_repo")

import numpy as np
import ml_dtypes

import concourse.bass as bass
import concourse.bacc as bacc
import concourse.tile as tile
import concourse.mybir as mybir
from concourse.bass_utils import run_bass_kernel_spmd





N_CORES = 8
N = 4096
DIM = 512
RPC = N // N_CORES          # rows per core = 512
NG = RPC // 128             # row groups per core = 4
NK = DIM // 128             # contraction tiles = 4
NF = 24                     # thresholds
XDVE = 9                    # thresholds computed on DVE+PE instead of ACT
TEMP = 0.15
C0 = 32.0                   # exp recentering constant (d range ~ [24, 41])
P = 128
HW = N // 2                 # 2048-wide half units
F32 = mybir.dt.float32
BF16 = mybir.dt.bfloat16
AF = mybir.ActivationFunctionType
ALU = mybir.AluOpType
AX = mybir.AxisListType
NPBF = ml_dtypes.bfloat16

_COMPILED = None
LAST_RESULTS = None


def _build():
    nc = bacc.Bacc("TRN2", target_bir_lowering=False, debug=False,
                   num_devices=N_CORES)

    mhi_d = nc.dram_tensor("mhi", [DIM, N], BF16, kind="ExternalInput")
    mlo_d = nc.dram_tensor("mlo", [DIM, N], BF16, kind="ExternalInput")
    whi_d = nc.dram_tensor("whi", [DIM, RPC], BF16, kind="ExternalInput")
    wlo_d = nc.dram_tensor("wlo", [DIM, RPC], BF16, kind="ExternalInput")
    sqc_d = nc.dram_tensor("sqc", [P, NG], F32, kind="ExternalInput")
    sqj_d = nc.dram_tensor("sqj", [P, N], F32, kind="ExternalInput")
    eye9_d = nc.dram_tensor("eye9", [P, P], F32, kind="ExternalInput")
    lin_d = nc.dram_tensor("lin", [P, 2 * NF], F32, kind="ExternalInput")

    deg_d = nc.dram_tensor("deg", [NG, P, NF], F32, kind="ExternalOutput")
    sumr_d = nc.dram_tensor("sumr", [XDVE, 512], F32, kind="ExternalOutput")
    mm_d = nc.dram_tensor("mm", [1, 8], F32, kind="ExternalOutput")

    cc_in = nc.dram_tensor("cc_in", [1, 8], F32)
    cc_ag = nc.dram_tensor("cc_ag", [N_CORES, 8], F32, addr_space="Shared")
    cc_warm = nc.dram_tensor("cc_warm", [N_CORES, 8], F32, addr_space="Shared")

    with tile.TileContext(nc) as tc:
        with (
            tc.tile_pool(name="persist", bufs=1) as pp,
            tc.tile_pool(name="psum", bufs=2, space="PSUM") as psum,
        ):
            # ---- loads (emission order ~ arrival priority) ----
            whit, wlot = [], []
            for k in range(NK):
                t = pp.tile([P, RPC], BF16, tag=f"whi{k}", name=f"whi{k}")
                nc.sync.dma_start(t[:], whi_d[k * P:(k + 1) * P, :])
                whit.append(t)
            sqc = pp.tile([P, NG], F32, tag="sqc")
            nc.sync.dma_start(sqc[:], sqc_d[:])
            mhit, mlot = [], []
            for k in range(NK):
                t = pp.tile([P, N], BF16, tag=f"big{k}", name=f"mhi{k}",
                            padded_shape=[P, N])
                mhit.append(t)
            for k in range(NK):
                t = pp.tile([P, N], BF16, tag=f"mlo{k}", name=f"mlo{k}")
                mlot.append(t)
            for k in range(NK):
                nc.sync.dma_start(mhit[k][:, 0:HW], mhi_d[k * P:(k + 1) * P, 0:HW])
            sqj = pp.tile([P, N], F32, tag="sqj")
            nc.sync.dma_start(sqj[:, 0:HW], sqj_d[:, 0:HW])
            for k in range(NK):
                nc.sync.dma_start(mlot[k][:, 0:HW], mlo_d[k * P:(k + 1) * P, 0:HW])
            for k in range(NK):
                t = pp.tile([P, RPC], BF16, tag=f"wlo{k}", name=f"wlo{k}")
                nc.sync.dma_start(t[:], wlo_d[k * P:(k + 1) * P, :])
                wlot.append(t)
            nc.sync.dma_start(sqj[:, HW:N], sqj_d[:, HW:N])
            for k in range(NK):
                nc.sync.dma_start(mhit[k][:, HW:N], mhi_d[k * P:(k + 1) * P, HW:N])
            for k in range(NK):
                nc.sync.dma_start(mlot[k][:, HW:N], mlo_d[k * P:(k + 1) * P, HW:N])
            eye9 = pp.tile([P, P], F32, tag="eye9")
            nc.sync.dma_start(eye9[:], eye9_d[:])
            lin = pp.tile([P, 2 * NF], F32, tag="lin")
            nc.sync.dma_start(lin[:], lin_d[:])

            # warm up the ncfw collective path early (saves ~20us on the
            # real AllGather later; result unused)
            warmsb = pp.tile([1, 8], F32, tag="warmsb")
            nc.gpsimd.memset(warmsb[:], 0.0)
            nc.gpsimd.dma_start(cc_in[:], warmsb[:])
            for _ in range(3):
                nc.gpsimd.collective_compute(
                    "AllGather", ALU.bypass,
                    replica_groups=[list(range(N_CORES))],
                    ins=[cc_in[:]], outs=[cc_warm[:]])

            ones128 = pp.tile([1, P], F32, tag="ones128")
            nc.vector.memset(ones128[:], 1.0)
            c0t = pp.tile([P, 1], F32, tag="c0t")
            nc.vector.memset(c0t[:], float(np.float32(C0) / np.float32(TEMP)))
            nc0t = pp.tile([P, 1], F32, tag="nc0t")
            nc.vector.memset(nc0t[:], float(np.float32(-C0) / np.float32(TEMP)))

            Dg = [pp.tile([P, N], F32, tag=f"D{g}", name=f"D{g}")
                  for g in range(NG)]
            ECg = None  # allocated after GEMM, reusing big{g} slots
            degt = [pp.tile([P, NF], F32, tag=f"deg{g}", name=f"degt{g}")
                    for g in range(NG)]
            for g in range(NG):
                nc.vector.memset(degt[g][:, 0:XDVE], 0.0)
            ones_col = pp.tile([P, 1], F32, tag="ones_col")
            nc.vector.memset(ones_col[:], 1.0)
            srow = pp.tile([1, 512], F32, tag="srow")
            maxp = pp.tile([P, NG * 2], F32, tag="maxp")
            minp = pp.tile([P, NG * 2], F32, tag="minp")

            # ---- GEMM (bf16 hi/lo x3) + d2 assembly + sqrt + min/max ----
            for h in range(2):
                for g in range(NG):
                    bank = psum.tile([P, HW], F32, tag="bank", name="bank")
                    for k in range(NK):          # whi . mhi
                        w = whit[k][:, g * P:(g + 1) * P]
                        for c in range(4):
                            nc.tensor.matmul(
                                bank[:, c * 512:(c + 1) * 512], w,
                                mhit[k][:, h * HW + c * 512:
                                      h * HW + (c + 1) * 512],
                                start=(k == 0), stop=False)
                    for k in range(NK):          # whi . mlo
                        w = whit[k][:, g * P:(g + 1) * P]
                        for c in range(4):
                            nc.tensor.matmul(
                                bank[:, c * 512:(c + 1) * 512], w,
                                mlot[k][:, h * HW + c * 512:
                                      h * HW + (c + 1) * 512],
                                start=False, stop=False)
                    for k in range(NK):          # wlo . mhi
                        w = wlot[k][:, g * P:(g + 1) * P]
                        for c in range(4):
                            nc.tensor.matmul(
                                bank[:, c * 512:(c + 1) * 512], w,
                                mhit[k][:, h * HW + c * 512:
                                         h * HW + (c + 1) * 512],
                                start=False, stop=(k == NK - 1))
                    # d2 = (psum + sq_i) + sq_j
                    nc.vector.scalar_tensor_tensor(
                        bank[:], bank[:], sqc[:, g:g + 1],
                        sqj[:, h * HW:(h + 1) * HW], ALU.add, ALU.add)
                    if h == 0:
                        # clamp the diag block (only place d2 can be < 0)
                        nc.vector.tensor_scalar(
                            bank[:, g * P:(g + 1) * P],
                            bank[:, g * P:(g + 1) * P], 0.0, None, ALU.max)
                    nc.scalar.activation(
                        Dg[g][:, h * HW:(h + 1) * HW], bank[:], AF.Sqrt)

                    u = g * 2 + h
                    half_ap = Dg[g][:, h * HW:(h + 1) * HW]
                    nc.vector.tensor_reduce(
                        maxp[:, u:u + 1], half_ap, axis=AX.X, op=ALU.max)
                    if h == 0:
                        nc.vector.tensor_tensor(
                            out=Dg[g][:, g * P:(g + 1) * P],
                            in0=Dg[g][:, g * P:(g + 1) * P],
                            in1=eye9[:], op=ALU.add)
                    nc.vector.tensor_reduce(
                        minp[:, u:u + 1], half_ap, axis=AX.X, op=ALU.min)

            # ---- EC = exp(-(D - C0)/TEMP) in bf16 (runs under PE/collective)
            ECg = [pp.tile([P, N], BF16, tag=f"big{g}", name=f"EC{g}")
                   for g in range(NG)]
            scl_exp = float(np.float32(-1.0) / np.float32(TEMP))
            for g in range(NG):
                for h in range(2):
                    nc.scalar.activation(
                        ECg[g][:, h * HW:(h + 1) * HW],
                        Dg[g][:, h * HW:(h + 1) * HW],
                        AF.Exp, bias=c0t[:], scale=scl_exp)

            # ---- global d_min/d_max: AllGather + cross-lane max ----
            mincol = pp.tile([P, 1], F32, tag="mincol")
            maxcol = pp.tile([P, 1], F32, tag="maxcol")
            nc.vector.tensor_reduce(mincol[:], minp[:], axis=AX.X, op=ALU.min)
            nc.vector.tensor_reduce(maxcol[:], maxp[:], axis=AX.X, op=ALU.max)
            mmpart = pp.tile([P, 2], F32, tag="mmpart")
            nc.vector.tensor_scalar(mmpart[:, 0:1], mincol[:], -1.0, None,
                                    ALU.mult)
            nc.vector.tensor_copy(mmpart[:, 1:2], maxcol[:])
            mmrow = pp.tile([1, 2], F32, tag="mmrow")
            nc.gpsimd.tensor_reduce(mmrow[:], mmpart[:], axis=AX.C, op=ALU.max)
            ccs = pp.tile([1, 8], F32, tag="ccs")
            nc.vector.memset(ccs[:], -3.0e38)
            nc.vector.tensor_copy(ccs[:, 0:2], mmrow[:])
            nc.gpsimd.dma_start(cc_in[:], ccs[:])
            nc.gpsimd.collective_compute(
                "AllGather", ALU.bypass,
                replica_groups=[list(range(N_CORES))],
                ins=[cc_in[:]], outs=[cc_ag[:]])
            agt = pp.tile([N_CORES, 8], F32, tag="agt")
            nc.gpsimd.dma_start(agt[:], cc_ag[:])
            mmrow2 = pp.tile([1, 8], F32, tag="mmrow2")
            nc.gpsimd.tensor_reduce(mmrow2[:], agt[:], axis=AX.C, op=ALU.max)
            nc.sync.dma_start(mm_d[:], mmrow2[:])

            # broadcast to all partitions via PE rank-1 (ones x row)
            pb = psum.tile([P, 8], F32, tag="bank", name="pbx")
            nc.tensor.matmul(pb[:], ones128[:], mmrow2[:], start=True,
                             stop=True)
            mmg = pp.tile([P, 8], F32, tag="mmg")
            nc.vector.tensor_copy(mmg[:], pb[:])

            # ---- thresholds (mirrors jnp.linspace fp32 semantics) ----
            dmin = pp.tile([P, 1], F32, tag="dmin")
            nc.vector.tensor_scalar(dmin[:], mmg[:, 0:1], -1.0, None, ALU.mult)
            dmin4 = pp.tile([P, 1], F32, tag="dmin4")
            nc.vector.tensor_scalar(dmin4[:], dmin[:], 1.0e-4, None, ALU.add)
            dmax = pp.tile([P, 1], F32, tag="dmax")
            nc.vector.tensor_tensor(out=dmax[:], in0=mmg[:, 1:2],
                                    in1=dmin4[:], op=ALU.max)
            ta = pp.tile([P, NF], F32, tag="ta")
            tb = pp.tile([P, NF], F32, tag="tb")
            thr = pp.tile([P, NF], F32, tag="thr")
            # t_k = d_min*(1-s_k) + d_max*s_k ; lin cols [0:NF]=s, [NF:]=1-s
            nc.vector.tensor_scalar(ta[:], lin[:, NF:2 * NF], dmin[:], None,
                                    ALU.mult)
            nc.vector.tensor_scalar(tb[:], lin[:, 0:NF], dmax[:], None,
                                    ALU.mult)
            nc.vector.tensor_tensor(out=thr[:], in0=ta[:], in1=tb[:],
                                    op=ALU.add)
            bias128 = pp.tile([P, NF], F32, tag="bias128")
            nc.vector.tensor_scalar(bias128[:], thr[:],
                                    float(np.float32(1.0) / np.float32(TEMP)),
                                    None, ALU.mult)
            # b_k = exp((t_k - C0)/TEMP) for the DVE sigmoid path
            b128 = pp.tile([P, NF], F32, tag="b128")
            nc.scalar.activation(
                b128[:], thr[:], AF.Exp, bias=nc0t[:],
                scale=float(np.float32(1.0) / np.float32(TEMP)))

            # ---- sigmoid passes ----
            # DVE path (k < XDVE): r = 1/(1 + EC*b_k); sum_j r -> sumr
            # (deg = N - sumr on host).  ACT path (k >= XDVE): accum_out.
            scl_sig = float(np.float32(-1.0) / np.float32(TEMP))
            tmpB = pp.tile([P, N], F32, tag="mlo0", name="tmpB")
            tmpB2 = pp.tile([P, N], F32, tag="mlo2", name="tmpB2")
            for k in range(XDVE):
                skb = psum.tile([1, 512], F32, tag="bank", name="skb")
                for g in range(NG):
                    buf = tmpB if (k * NG + g) % 2 == 0 else tmpB2
                    nc.vector.tensor_scalar(
                        buf[:], ECg[g][:], b128[:, k:k + 1], 1.0,
                        ALU.mult, ALU.add)
                    nc.vector.reciprocal_approx_fast(buf[:], buf[:])
                    for c in range(8):
                        nc.tensor.matmul(
                            skb[:], ones_col[:],
                            buf[:, c * 512:(c + 1) * 512],
                            start=(g == 0 and c == 0),
                            stop=(g == NG - 1 and c == 7))
                nc.vector.tensor_copy(srow[:], skb[:])
                nc.sync.dma_start(sumr_d[k:k + 1, :], srow[:])
            for k in range(XDVE, NF):
                for g in range(NG):
                    scr = pp.tile([P, N], BF16, tag="mlo1", name="scr")
                    nc.scalar.activation(
                        scr[:], Dg[g][:], AF.Sigmoid,
                        bias=bias128[:, k:k + 1], scale=scl_sig,
                        accum_out=degt[g][:, k:k + 1])

            for g in range(NG):
                nc.sync.dma_start(deg_d[g], degt[g][:])

    nc.compile()
    return nc


def _get_compiled():
    global _COMPILED
    if _COMPILED is None:
        _COMPILED = (_build(),)
    return _COMPILED[0]


def make_in_maps(embeddings: np.ndarray):
    emb = np.ascontiguousarray(np.asarray(embeddings, dtype=np.float32))
    assert emb.shape == (N, DIM)
    embT = np.ascontiguousarray(emb.T)                      # [512, 4096]
    m2 = np.ascontiguousarray(-2.0 * embT)
    sq = (emb.astype(np.float64) ** 2).sum(axis=1).astype(np.float32)

    s = (np.arange(NF, dtype=np.float32) / np.float32(NF - 1)).astype(np.float32)
    s[NF - 1] = 1.0
    oms = (np.float32(1.0) - s).astype(np.float32)
    lin = np.broadcast_to(np.concatenate([s, oms]).reshape(1, 2 * NF),
                          (P, 2 * NF))
    lin = np.ascontiguousarray(lin, dtype=np.float32)
    eye9 = (np.eye(P, dtype=np.float32) * np.float32(1e9))

    in_maps = []
    for c in range(N_CORES):
        lo, hi = c * RPC, (c + 1) * RPC
        perm = np.concatenate([np.arange(lo, hi), np.arange(0, lo),
                               np.arange(hi, N)])
        mp = m2[:, perm]
        mhi = mp.astype(NPBF)
        mlo = (mp - mhi.astype(np.float32)).astype(NPBF)
        wp = embT[:, lo:hi]
        whi = wp.astype(NPBF)
        wlo = (wp - whi.astype(np.float32)).astype(NPBF)
        sqjp = np.ascontiguousarray(
            np.broadcast_to(sq[perm].reshape(1, N), (P, N)), dtype=np.float32)
        in_maps.append({
            "mhi": np.ascontiguousarray(mhi),
            "mlo": np.ascontiguousarray(mlo),
            "whi": np.ascontiguousarray(whi),
            "wlo": np.ascontiguousarray(wlo),
            "sqc": np.ascontiguousarray(sq[lo:hi].reshape(NG, P).T),
            "sqj": sqjp,
            "eye9": eye9,
            "lin": lin,
        })
    return in_maps


def finalize(deg_blocks, sumr_blocks) -> np.float32:
    """deg_blocks: [NG,P,NF] per core (ACT cols valid for k>=XDVE);
    sumr_blocks: [XDVE,512] per core (PE block sums of r = 1-sigma).
    h0 for k<XDVE is identically 0 in the loss (only h0[-8:] is used)."""
    deg = np.concatenate([d.reshape(RPC, NF) for d in deg_blocks], axis=0)
    degc = np.maximum(deg, np.float32(1e-6))
    h0 = (degc < 0.5).sum(axis=0).astype(np.float64)        # [24]
    h0[:XDVE] = 0.0
    S = deg.astype(np.float64).sum(axis=0)                  # [24]
    sumr_tot = np.stack([s.astype(np.float64).sum(axis=1)
                         for s in sumr_blocks]).sum(axis=0)  # [XDVE]
    S[:XDVE] = float(N) * float(N) - sumr_tot
    n_excess = np.maximum(S / 2.0 - (N - 1), 0.0) / N
    h0_loss = h0[-8:].mean()
    h1_loss = n_excess.mean()
    total = (h0_loss + 0.5 * h1_loss) * 0.1
    return np.float32(total)


def kernel(**inputs) -> np.ndarray:
    global LAST_RESULTS
    emb = inputs["embeddings"]
    nc = _get_compiled()
    in_maps = make_in_maps(emb)
    res = run_bass_kernel_spmd(nc, in_maps, list(range(N_CORES)))
    LAST_RESULTS = res
    out = finalize([res.results[c]["deg"] for c in range(N_CORES)],
                   [res.results[c]["sumr"] for c in range(N_CORES)])
    return np.asarray(out, dtype=np.float32)


if __name__ == "__main__":
    rng = np.random.default_rng(0)
    emb = rng.standard_normal((N, DIM)).astype(np.float32)
    print(kernel(embeddings=emb, step=0))


# revision 22
# speedup vs baseline: 1.0149x; 1.0149x over previous
"""PersistenceLandscapeLoss on 8 TRN2 NeuronCores via Bass/Tile.

Math (reference):
  D[i,j] = ||e_i - e_j||          (i != j; diag pushed to 1e9)
  d_min/d_max = min/max off-diag; thresholds = linspace(d_min, max(d_max, d_min+1e-4), 24)
  per threshold t: adj = sigmoid((t - D)/0.15) (zero diag); deg_i = row sums
  h0_t = #(deg_i < 0.5); S_t = sum(adj); n_excess_t = relu(S_t/2 - (N-1))/N
  loss = (mean(h0[-8:]) + 0.5*mean(n_excess)) * 0.1

Sharding: 512 distance-matrix rows per core; columns permuted per-core so the
diagonal block is at a static position (core's own columns first).
Per core:
  - GEMM on PE in bf16 hi/lo split (3 passes ~ fp32 accuracy, 2.5x faster
    than native fp32 matmul): psum = -2*G; DVE adds sq_i + sq_j; ACT sqrt.
  - EC = exp(-(D-c0)/TEMP) (bf16) precomputed on ACT during the GEMM phase;
    used by the DVE sigmoid path (sigmoid(x) = 1 - 1/(1+e^x)).
  - row min/max on DVE; AllGather + cross-lane max for global d_min/d_max;
    thresholds on-device mirroring jnp.linspace fp32 semantics.
  - 24 thresholds: first XDVE on DVE (mul+add, reciprocal_approx_fast,
    reduce), rest on ACT Sigmoid with accum_out. The two engines run
    concurrently.
Host gathers per-row degree partials and finishes the scalar reduction.
"""
import sys

if "/opt/trn_rl_repo" not in sys.path:
    sys.path.insert(0, "/opt/trn_rl_repo")

import numpy as np
import ml_dtypes

import concourse.bass as bass
import concourse.bacc as bacc
import concourse.tile as tile
import concourse.mybir as mybir
from concourse.bass_utils import run_bass_kernel_spmd





N_CORES = 8
N = 4096
DIM = 512
RPC = N // N_CORES          # rows per core = 512
NG = RPC // 128             # row groups per core = 4
NK = DIM // 128             # contraction tiles = 4
NF = 24                     # thresholds
XDVE = 7                    # thresholds computed on DVE+PE instead of ACT
TEMP = 0.15
C0 = 32.0                   # exp recentering constant (d range ~ [24, 41])
P = 128
HW = N // 2                 # 2048-wide half units
F32 = mybir.dt.float32
BF16 = mybir.dt.bfloat16
AF = mybir.ActivationFunctionType
ALU = mybir.AluOpType
AX = mybir.AxisListType
NPBF = ml_dtypes.bfloat16

_COMPILED = None
LAST_RESULTS = None


def _build():
    nc = bacc.Bacc("TRN2", target_bir_lowering=False, debug=False,
                   num_devices=N_CORES)

    mhi_d = nc.dram_tensor("mhi", [DIM, N], BF16, kind="ExternalInput")
    mlo_d = nc.dram_tensor("mlo", [DIM, N], BF16, kind="ExternalInput")
    whi_d = nc.dram_tensor("whi", [DIM, RPC], BF16, kind="ExternalInput")
    wlo_d = nc.dram_tensor("wlo", [DIM, RPC], BF16, kind="ExternalInput")
    sqc_d = nc.dram_tensor("sqc", [P, NG], F32, kind="ExternalInput")
    sqj_d = nc.dram_tensor("sqj", [P, N], F32, kind="ExternalInput")
    eye9_d = nc.dram_tensor("eye9", [P, P], F32, kind="ExternalInput")
    lin_d = nc.dram_tensor("lin", [P, 2 * NF], F32, kind="ExternalInput")

    deg_d = nc.dram_tensor("deg", [NG, P, NF], F32, kind="ExternalOutput")
    sumr_d = nc.dram_tensor("sumr", [XDVE, 512], F32, kind="ExternalOutput")
    mm_d = nc.dram_tensor("mm", [1, 8], F32, kind="ExternalOutput")

    cc_in = nc.dram_tensor("cc_in", [1, 8], F32)
    cc_ag = nc.dram_tensor("cc_ag", [N_CORES, 8], F32, addr_space="Shared")
    cc_warm = nc.dram_tensor("cc_warm", [N_CORES, 8], F32, addr_space="Shared")

    with tile.TileContext(nc) as tc:
        with (
            tc.tile_pool(name="persist", bufs=1) as pp,
            tc.tile_pool(name="psum", bufs=2, space="PSUM") as psum,
        ):
            # ---- loads (emission order ~ arrival priority) ----
            whit, wlot = [], []
            for k in range(NK):
                t = pp.tile([P, RPC], BF16, tag=f"whi{k}", name=f"whi{k}")
                nc.sync.dma_start(t[:], whi_d[k * P:(k + 1) * P, :])
                whit.append(t)
            sqc = pp.tile([P, NG], F32, tag="sqc")
            nc.sync.dma_start(sqc[:], sqc_d[:])
            mhit, mlot = [], []
            for k in range(NK):
                t = pp.tile([P, N], BF16, tag=f"big{k}", name=f"mhi{k}",
                            padded_shape=[P, N])
                mhit.append(t)
            for k in range(NK):
                t = pp.tile([P, N], BF16, tag=f"mlo{k}", name=f"mlo{k}")
                mlot.append(t)
            for k in range(NK):
                nc.sync.dma_start(mhit[k][:, 0:HW], mhi_d[k * P:(k + 1) * P, 0:HW])
            sqj = pp.tile([P, N], F32, tag="sqj")
            nc.sync.dma_start(sqj[:, 0:HW], sqj_d[:, 0:HW])
            for k in range(NK):
                nc.sync.dma_start(mlot[k][:, 0:HW], mlo_d[k * P:(k + 1) * P, 0:HW])
            for k in range(NK):
                t = pp.tile([P, RPC], BF16, tag=f"wlo{k}", name=f"wlo{k}")
                nc.sync.dma_start(t[:], wlo_d[k * P:(k + 1) * P, :])
                wlot.append(t)
            nc.sync.dma_start(sqj[:, HW:N], sqj_d[:, HW:N])
            for k in range(NK):
                nc.sync.dma_start(mhit[k][:, HW:N], mhi_d[k * P:(k + 1) * P, HW:N])
            for k in range(NK):
                nc.sync.dma_start(mlot[k][:, HW:N], mlo_d[k * P:(k + 1) * P, HW:N])
            eye9 = pp.tile([P, P], F32, tag="eye9")
            nc.sync.dma_start(eye9[:], eye9_d[:])
            lin = pp.tile([P, 2 * NF], F32, tag="lin")
            nc.sync.dma_start(lin[:], lin_d[:])

            # warm up the ncfw collective path early (saves ~20us on the
            # real AllGather later; result unused)
            warmsb = pp.tile([1, 8], F32, tag="warmsb")
            nc.gpsimd.memset(warmsb[:], 0.0)
            nc.gpsimd.dma_start(cc_in[:], warmsb[:])
            for _ in range(3):
                nc.gpsimd.collective_compute(
                    "AllGather", ALU.bypass,
                    replica_groups=[list(range(N_CORES))],
                    ins=[cc_in[:]], outs=[cc_warm[:]])

            ones128 = pp.tile([1, P], F32, tag="ones128")
            nc.vector.memset(ones128[:], 1.0)
            c0t = pp.tile([P, 1], F32, tag="c0t")
            nc.vector.memset(c0t[:], float(np.float32(C0) / np.float32(TEMP)))
            nc0t = pp.tile([P, 1], F32, tag="nc0t")
            nc.vector.memset(nc0t[:], float(np.float32(-C0) / np.float32(TEMP)))

            Dg = [pp.tile([P, N], F32, tag=f"D{g}", name=f"D{g}")
                  for g in range(NG)]
            ECg = None  # allocated after GEMM, reusing big{g} slots
            degt = [pp.tile([P, NF], F32, tag=f"deg{g}", name=f"degt{g}")
                    for g in range(NG)]
            for g in range(NG):
                nc.vector.memset(degt[g][:, 0:XDVE], 0.0)
            ones_col = pp.tile([P, 1], BF16, tag="ones_col")
            nc.vector.memset(ones_col[:], 1.0)
            srow = pp.tile([1, 512], F32, tag="srow")
            maxp = pp.tile([P, NG * 2], F32, tag="maxp")
            minp = pp.tile([P, NG * 2], F32, tag="minp")

            # ---- GEMM (bf16 hi/lo x3) + d2 assembly + sqrt + min/max ----
            for h in range(2):
                for g in range(NG):
                    bank = psum.tile([P, HW], F32, tag="bank", name="bank")
                    for k in range(NK):          # whi . mhi
                        w = whit[k][:, g * P:(g + 1) * P]
                        for c in range(4):
                            nc.tensor.matmul(
                                bank[:, c * 512:(c + 1) * 512], w,
                                mhit[k][:, h * HW + c * 512:
                                      h * HW + (c + 1) * 512],
                                start=(k == 0), stop=False)
                    for k in range(NK):          # whi . mlo
                        w = whit[k][:, g * P:(g + 1) * P]
                        for c in range(4):
                            nc.tensor.matmul(
                                bank[:, c * 512:(c + 1) * 512], w,
                                mlot[k][:, h * HW + c * 512:
                                      h * HW + (c + 1) * 512],
                                start=False, stop=False)
                    for k in range(NK):          # wlo . mhi
                        w = wlot[k][:, g * P:(g + 1) * P]
                        for c in range(4):
                            nc.tensor.matmul(
                                bank[:, c * 512:(c + 1) * 512], w,
                                mhit[k][:, h * HW + c * 512:
                                         h * HW + (c + 1) * 512],
                                start=False, stop=(k == NK - 1))
                    # d2 = (psum + sq_i) + sq_j
                    nc.vector.scalar_tensor_tensor(
                        bank[:], bank[:], sqc[:, g:g + 1],
                        sqj[:, h * HW:(h + 1) * HW], ALU.add, ALU.add)
                    if h == 0:
                        # clamp the diag block (only place d2 can be < 0)
                        nc.vector.tensor_scalar(
                            bank[:, g * P:(g + 1) * P],
                            bank[:, g * P:(g + 1) * P], 0.0, None, ALU.max)
                    nc.scalar.activation(
                        Dg[g][:, h * HW:(h + 1) * HW], bank[:], AF.Sqrt)

                    u = g * 2 + h
                    half_ap = Dg[g][:, h * HW:(h + 1) * HW]
                    nc.vector.tensor_reduce(
                        maxp[:, u:u + 1], half_ap, axis=AX.X, op=ALU.max)
                    if h == 0:
                        nc.vector.tensor_tensor(
                            out=Dg[g][:, g * P:(g + 1) * P],
                            in0=Dg[g][:, g * P:(g + 1) * P],
                            in1=eye9[:], op=ALU.add)
                    nc.vector.tensor_reduce(
                        minp[:, u:u + 1], half_ap, axis=AX.X, op=ALU.min)

            # ---- EC = exp(-(D - C0)/TEMP) in bf16 (runs under PE/collective)
            ECg = [pp.tile([P, N], BF16, tag=f"big{g}", name=f"EC{g}")
                   for g in range(NG)]
            scl_exp = float(np.float32(-1.0) / np.float32(TEMP))
            for g in range(NG):
                for h in range(2):
                    nc.scalar.activation(
                        ECg[g][:, h * HW:(h + 1) * HW],
                        Dg[g][:, h * HW:(h + 1) * HW],
                        AF.Exp, bias=c0t[:], scale=scl_exp)

            # ---- global d_min/d_max: AllGather + cross-lane max ----
            mincol = pp.tile([P, 1], F32, tag="mincol")
            maxcol = pp.tile([P, 1], F32, tag="maxcol")
            nc.vector.tensor_reduce(mincol[:], minp[:], axis=AX.X, op=ALU.min)
            nc.vector.tensor_reduce(maxcol[:], maxp[:], axis=AX.X, op=ALU.max)
            mmpart = pp.tile([P, 2], F32, tag="mmpart")
            nc.vector.tensor_scalar(mmpart[:, 0:1], mincol[:], -1.0, None,
                                    ALU.mult)
            nc.vector.tensor_copy(mmpart[:, 1:2], maxcol[:])
            mmrow = pp.tile([1, 2], F32, tag="mmrow")
            nc.gpsimd.tensor_reduce(mmrow[:], mmpart[:], axis=AX.C, op=ALU.max)
            ccs = pp.tile([1, 8], F32, tag="ccs")
            nc.vector.memset(ccs[:], -3.0e38)
            nc.vector.tensor_copy(ccs[:, 0:2], mmrow[:])
            nc.gpsimd.dma_start(cc_in[:], ccs[:])
            nc.gpsimd.collective_compute(
                "AllGather", ALU.bypass,
                replica_groups=[list(range(N_CORES))],
                ins=[cc_in[:]], outs=[cc_ag[:]])
            agt = pp.tile([N_CORES, 8], F32, tag="agt")
            nc.gpsimd.dma_start(agt[:], cc_ag[:])
            mmrow2 = pp.tile([1, 8], F32, tag="mmrow2")
            nc.gpsimd.tensor_reduce(mmrow2[:], agt[:], axis=AX.C, op=ALU.max)
            nc.sync.dma_start(mm_d[:], mmrow2[:])

            # broadcast to all partitions via PE rank-1 (ones x row)
            pb = psum.tile([P, 8], F32, tag="bank", name="pbx")
            nc.tensor.matmul(pb[:], ones128[:], mmrow2[:], start=True,
                             stop=True)
            mmg = pp.tile([P, 8], F32, tag="mmg")
            nc.vector.tensor_copy(mmg[:], pb[:])

            # ---- thresholds (mirrors jnp.linspace fp32 semantics) ----
            dmin = pp.tile([P, 1], F32, tag="dmin")
            nc.vector.tensor_scalar(dmin[:], mmg[:, 0:1], -1.0, None, ALU.mult)
            dmin4 = pp.tile([P, 1], F32, tag="dmin4")
            nc.vector.tensor_scalar(dmin4[:], dmin[:], 1.0e-4, None, ALU.add)
            dmax = pp.tile([P, 1], F32, tag="dmax")
            nc.vector.tensor_tensor(out=dmax[:], in0=mmg[:, 1:2],
                                    in1=dmin4[:], op=ALU.max)
            ta = pp.tile([P, NF], F32, tag="ta")
            tb = pp.tile([P, NF], F32, tag="tb")
            thr = pp.tile([P, NF], F32, tag="thr")
            # t_k = d_min*(1-s_k) + d_max*s_k ; lin cols [0:NF]=s, [NF:]=1-s
            nc.vector.tensor_scalar(ta[:], lin[:, NF:2 * NF], dmin[:], None,
                                    ALU.mult)
            nc.vector.tensor_scalar(tb[:], lin[:, 0:NF], dmax[:], None,
                                    ALU.mult)
            nc.vector.tensor_tensor(out=thr[:], in0=ta[:], in1=tb[:],
                                    op=ALU.add)
            bias128 = pp.tile([P, NF], F32, tag="bias128")
            nc.vector.tensor_scalar(bias128[:], thr[:],
                                    float(np.float32(1.0) / np.float32(TEMP)),
                                    None, ALU.mult)
            # b_k = exp((t_k - C0)/TEMP) for the DVE sigmoid path
            b128 = pp.tile([P, NF], F32, tag="b128")
            nc.scalar.activation(
                b128[:], thr[:], AF.Exp, bias=nc0t[:],
                scale=float(np.float32(1.0) / np.float32(TEMP)))

            # ---- sigmoid passes ----
            # DVE path (k < XDVE): r = 1/(1 + EC*b_k); sum_j r -> sumr
            # (deg = N - sumr on host).  ACT path (k >= XDVE): accum_out.
            scl_sig = float(np.float32(-1.0) / np.float32(TEMP))
            tmpB = pp.tile([P, N], F32, tag="mlo0", name="tmpB")
            rbf = pp.tile([P, N], BF16, tag="sqj", name="rbf")
            for k in range(XDVE):
                skb = psum.tile([1, 512], F32, tag="bank", name="skb")
                for g in range(NG):
                    nc.vector.tensor_scalar(
                        tmpB[:], ECg[g][:], b128[:, k:k + 1], 1.0,
                        ALU.mult, ALU.add)
                    nc.vector.reciprocal_approx_fast(tmpB[:], tmpB[:])
                    nc.vector.tensor_copy(rbf[:], tmpB[:])
                    for c in range(8):
                        nc.tensor.matmul(
                            skb[:], ones_col[:],
                            rbf[:, c * 512:(c + 1) * 512],
                            start=(g == 0 and c == 0),
                            stop=(g == NG - 1 and c == 7))
                nc.vector.tensor_copy(srow[:], skb[:])
                nc.sync.dma_start(sumr_d[k:k + 1, :], srow[:])
            for k in range(XDVE, NF):
                for g in range(NG):
                    scr = pp.tile([P, N], BF16, tag="mlo1", name="scr")
                    nc.scalar.activation(
                        scr[:], Dg[g][:], AF.Sigmoid,
                        bias=bias128[:, k:k + 1], scale=scl_sig,
                        accum_out=degt[g][:, k:k + 1])

            for g in range(NG):
                nc.sync.dma_start(deg_d[g], degt[g][:])

    nc.compile()
    return nc


def _get_compiled():
    global _COMPILED
    if _COMPILED is None:
        _COMPILED = (_build(),)
    return _COMPILED[0]


def make_in_maps(embeddings: np.ndarray):
    emb = np.ascontiguousarray(np.asarray(embeddings, dtype=np.float32))
    assert emb.shape == (N, DIM)
    embT = np.ascontiguousarray(emb.T)                      # [512, 4096]
    m2 = np.ascontiguousarray(-2.0 * embT)
    sq = (emb.astype(np.float64) ** 2).sum(axis=1).astype(np.float32)

    s = (np.arange(NF, dtype=np.float32) / np.float32(NF - 1)).astype(np.float32)
    s[NF - 1] = 1.0
    oms = (np.float32(1.0) - s).astype(np.float32)
    lin = np.broadcast_to(np.concatenate([s, oms]).reshape(1, 2 * NF),
                          (P, 2 * NF))
    lin = np.ascontiguousarray(lin, dtype=np.float32)
    eye9 = (np.eye(P, dtype=np.float32) * np.float32(1e9))

    in_maps = []
    for c in range(N_CORES):
        lo, hi = c * RPC, (c + 1) * RPC
        perm = np.concatenate([np.arange(lo, hi), np.arange(0, lo),
                               np.arange(hi, N)])
        mp = m2[:, perm]
        mhi = mp.astype(NPBF)
        mlo = (mp - mhi.astype(np.float32)).astype(NPBF)
        wp = embT[:, lo:hi]
        whi = wp.astype(NPBF)
        wlo = (wp - whi.astype(np.float32)).astype(NPBF)
        sqjp = np.ascontiguousarray(
            np.broadcast_to(sq[perm].reshape(1, N), (P, N)), dtype=np.float32)
        in_maps.append({
            "mhi": np.ascontiguousarray(mhi),
            "mlo": np.ascontiguousarray(mlo),
            "whi": np.ascontiguousarray(whi),
            "wlo": np.ascontiguousarray(wlo),
            "sqc": np.ascontiguousarray(sq[lo:hi].reshape(NG, P).T),
            "sqj": sqjp,
            "eye9": eye9,
            "lin": lin,
        })
    return in_maps


def finalize(deg_blocks, sumr_blocks) -> np.float32:
    """deg_blocks: [NG,P,NF] per core (ACT cols valid for k>=XDVE);
    sumr_blocks: [XDVE,512] per core (PE block sums of r = 1-sigma).
    h0 for k<XDVE is identically 0 in the loss (only h0[-8:] is used)."""
    deg = np.concatenate([d.reshape(RPC, NF) for d in deg_blocks], axis=0)
    degc = np.maximum(deg, np.float32(1e-6))
    h0 = (degc < 0.5).sum(axis=0).astype(np.float64)        # [24]
    h0[:XDVE] = 0.0
    S = deg.astype(np.float64).sum(axis=0)                  # [24]
    sumr_tot = np.stack([s.astype(np.float64).sum(axis=1)
                         for s in sumr_blocks]).sum(axis=0)  # [XDVE]
    S[:XDVE] = float(N) * float(N) - sumr_tot
    n_excess = np.maximum(S / 2.0 - (N - 1), 0.0) / N
    h0_loss = h0[-8:].mean()
    h1_loss = n_excess.mean()
    total = (h0_loss + 0.5 * h1_loss) * 0.1
    return np.float32(total)


def kernel(**inputs) -> np.ndarray:
    global LAST_RESULTS
    emb = inputs["embeddings"]
    nc = _get_compiled()
    in_maps = make_in_maps(emb)
    res = run_bass_kernel_spmd(nc, in_maps, list(range(N_CORES)))
    LAST_RESULTS = res
    out = finalize([res.results[c]["deg"] for c in range(N_CORES)],
                   [res.results[c]["sumr"] for c in range(N_CORES)])
    return np.asarray(out, dtype=np.float32)


if __name__ == "__main__":
    rng = np.random.default_rng(0)
    emb = rng.standard_normal((N, DIM)).astype(np.float32)
    print(kernel(embeddings=emb, step=0))
